# revision 59
# baseline (speedup 1.0000x reference)
"""Multi-head attention (B=2, S=2048, DM=1024, H=16, DH=64, causal) on 8 TRN2 cores.

Sharding: tensor-parallel over heads. Core c owns heads {2c, 2c+1} = q/k/v dims
[128c, 128c+128). Each core computes its QKV projections, causal attention for
its 2 heads (both batches), and a partial output projection (row-parallel over
Wo). Host unshards by summing the 8 partials and adding bo (the TP all-reduce).

Design (v3 — AV sum fusion + full-mode AV):
  - xT DMA'd in 8 seq-chunks over 4 queues so QKV compute starts after ~0.5MB.
  - QKV chunks interleaved with attention blocks.
  - Scores: 2 heads row-packed on the PE (K=64 each, concurrent), f32 PSUM.
  - V stored as V2a=[Vh0|ones], V2b=[Vh1|ones] (128 seq, 65): each head's AV
    matmul is M=65 full 128x128 mode (no col tiling -> LDWEIGHTS can overlap
    via the background weight buffer) and row 64 accumulates the softmax
    denominator for free. No separate sum matmul.
  - Reciprocal via DVE reciprocal_approx_fast, bounced through a pre-memset
    f32 tile (custom DVE ops need base partition 0 and can't read PSUM).
  - rec broadcast across partitions via two K=1 PE matmuls in disjoint
    row+col groups (concurrent).
  - Diagonal causal mask via GpSimd affine_select on the exp tile.
  - Tail (stash/recip/bcast/mult/outproj) software-pipelined into the NEXT
    attention block's emission so no engine drains.

Causality hardcoded (the reference's attention_mask is always triu causal).
"""

import os
import sys

import numpy as np

try:
    import concourse  # noqa: F401
except ImportError:
    sys.path.insert(0, "/opt/trn_rl_repo")

import ml_dtypes

BF16 = ml_dtypes.bfloat16

B, S, DM = 2, 2048, 1024
H, DH = 16, 64
NCORES = 8
CPC = DM // NCORES  # 128 q/k/v dims per core (2 heads)
BS = B * S  # 4096
Q_W = 512  # query-block width
N_CH = BS // Q_W  # 8 seq chunks
KT_FEAT = DM // 128  # 8 contraction tiles for QKV
NQB = S // Q_W  # 4 query blocks per batch

_CACHE = {}
LAST_EXEC_NS = None
LAST_RESULTS = None


def _build(repeat=1):
    import concourse.mybir as mybir
    from concourse import bacc
    from concourse import tile
    from concourse.masks import make_identity

    f32 = mybir.dt.float32
    f16 = mybir.dt.float16
    bf16 = mybir.dt.bfloat16
    Exp = mybir.ActivationFunctionType.Exp
    Ident = mybir.ActivationFunctionType.Identity
    is_ge = mybir.AluOpType.is_ge

    nc = bacc.Bacc(
        "TRN2",
        target_bir_lowering=False,
        debug=False,
        enable_asserts=False,
        num_devices=NCORES,
    )

    # host supplies xT pre-chunked contiguous: (chunk, 128, feat-tile, 512)
    xT = nc.dram_tensor("xT", (N_CH, 128, KT_FEAT, Q_W), bf16,
                        kind="ExternalInput").ap()
    # weights pre-rearranged host-side to (128, feat-tile, CPC) contiguous
    wq = nc.dram_tensor("wq", (128, KT_FEAT, CPC), bf16,
                        kind="ExternalInput").ap()
    wk = nc.dram_tensor("wk", (128, KT_FEAT, CPC), bf16,
                        kind="ExternalInput").ap()
    wv = nc.dram_tensor("wv", (128, KT_FEAT, CPC), bf16,
                        kind="ExternalInput").ap()
    wo = nc.dram_tensor("wo", (CPC, DM), bf16, kind="ExternalInput").ap()
    bq = nc.dram_tensor("bq", (CPC, 1), f32, kind="ExternalInput").ap()
    out = nc.dram_tensor("out", (BS, DM), f16, kind="ExternalOutput").ap()

    with tile.TileContext(nc) as tc:
      with tc.tile_pool(name="consts", bufs=1) as consts, \
           tc.tile_pool(name="sb", bufs=2) as sb, \
           tc.tile_pool(name="psp", bufs=1, space="PSUM") as psp:

        def body():
            # ---- persistent tiles ------------------------------------------
            wq_sb = consts.tile((128, KT_FEAT, CPC), bf16, name="wq_sb")
            wk_sb = consts.tile((128, KT_FEAT, CPC), bf16, name="wk_sb")
            wv_sb = consts.tile((128, KT_FEAT, CPC), bf16, name="wv_sb")
            wo_sb = consts.tile((CPC, DM), bf16, name="wo_sb")
            bq_sb = consts.tile((CPC, 1), f32, name="bq_sb")

            ident_sb = consts.tile((128, 128), bf16, name="ident_sb")
            make_identity(nc, ident_sb[:])
            ones_bf = consts.tile((128, 64), bf16, name="ones_bf")
            nc.vector.memset(ones_bf[:], 1.0)

            QT_sb = consts.tile((128, BS), bf16, name="QT_sb")
            KT_sb = consts.tile((128, BS), bf16, name="KT_sb")
            # V2: (seq 128, 32 k-tiles, 130) = [Vh0 d0-63 | ones | Vh1 | ones]
            # -> AV lhsT [0:65] / [65:130] gives each head [V|ones] (M=65,
            # full-mode matmul whose row 64 accumulates the softmax denom).
            V2_sb = consts.tile((128, BS // 128, 130), bf16, name="V2_sb")
            nc.vector.memset(V2_sb[:, :, 64:65], 1.0)
            nc.vector.memset(V2_sb[:, :, 129:130], 1.0)

            # xT in 8 per-chunk tiles so QKV(ch) only waits its own slab.
            # Startup: wq halves land on two queues while chunk 0 streams in
            # quarters over 4 queues -> first Q-proj matmul fires ~5us in.
            xts = [consts.tile((128, KT_FEAT, Q_W), bf16, name=f"xt{ch}")
                   for ch in range(N_CH)]
            # startup DMA: wq / chunk-0 / wk / wv slabs interleaved across the
            # three queues in PE consumption order, so the first Q-proj
            # matmul fires as soon as wq+first slab land (~4us)
            nc.sync.dma_start(bq_sb[:], bq)
            nc.scalar.dma_start(wq_sb[:, 0:3], wq[:, 0:3])
            nc.gpsimd.dma_start(wq_sb[:, 3:6], wq[:, 3:6])
            nc.sync.dma_start(wq_sb[:, 6:8], wq[:, 6:8])
            nc.sync.dma_start(xts[0][:, 0:1], xT[0, :, 0:1])
            nc.scalar.dma_start(xts[0][:, 1:2], xT[0, :, 1:2])
            nc.gpsimd.dma_start(xts[0][:, 2:3], xT[0, :, 2:3])
            nc.sync.dma_start(xts[0][:, 3:5], xT[0, :, 3:5])
            nc.scalar.dma_start(wk_sb[:, 0:3], wk[:, 0:3])
            nc.gpsimd.dma_start(wk_sb[:, 3:6], wk[:, 3:6])
            nc.sync.dma_start(wk_sb[:, 6:8], wk[:, 6:8])
            nc.scalar.dma_start(xts[0][:, 5:6], xT[0, :, 5:6])
            nc.gpsimd.dma_start(xts[0][:, 6:8], xT[0, :, 6:8])
            nc.scalar.dma_start(wv_sb[:, 0:3], wv[:, 0:3])
            nc.gpsimd.dma_start(wv_sb[:, 3:6], wv[:, 3:6])
            nc.sync.dma_start(wv_sb[:, 6:8], wv[:, 6:8])
            nc.scalar.dma_start(xts[1][:, 0:4], xT[1, :, 0:4])
            nc.gpsimd.dma_start(xts[1][:, 4:8], xT[1, :, 4:8])
            nc.sync.dma_start(wo_sb[:], wo)
            dma_engs = [nc.sync, nc.scalar, nc.gpsimd]
            for ch in range(2, N_CH):
                dma_engs[ch % 3].dma_start(xts[ch][:], xT[ch])

            # persistent pre-memset recip staging (ping-pong): full-tile
            # reciprocal_approx_fast needs every partition initialized, and
            # memset in the per-block chain would delay PSUM release
            stis = []
            for i in range(2):
                sti = consts.tile((128, Q_W), f32, name=f"sti{i}")
                nc.vector.memset(sti[:], 1.0)
                stis.append(sti)

            # ---- QKV chunk -------------------------------------------------
            def emit_qkv(ch):
                c0 = ch * Q_W
                vt = sb.tile((128, Q_W), bf16, name=f"vt{ch}", tag="vt",
                             bufs=2)
                # bias folds: scores use (Q+bq)@K ((Q+bq)@bk is constant over
                # keys -> softmax-invariant), and bv folds into bo host-side
                # (sum of attn weights is 1). Only Q needs its bias; K/V
                # evacuation becomes plain DVE copies, freeing ScalarE (the
                # attention-phase pacing engine) for exp.
                for pname, w_sb in (
                    ("q", wq_sb),
                    ("k", wk_sb),
                    ("v", wv_sb),
                ):
                    ps_p = psp.tile((128, Q_W), f32, name=f"ps_{pname}{ch}",
                                    tag="s", bufs=2)
                    for t in range(KT_FEAT):
                        nc.tensor.matmul(
                            ps_p[:],
                            lhsT=w_sb[:, t, :],
                            rhs=xts[ch][:, t, :],
                            start=(t == 0),
                            stop=(t == KT_FEAT - 1),
                        )
                    if pname == "q":
                        nc.scalar.activation(QT_sb[:, c0:c0 + Q_W], ps_p[:],
                                             Ident, bias=bq_sb[:])
                    elif pname == "k":
                        nc.vector.tensor_copy(KT_sb[:, c0:c0 + Q_W], ps_p[:])
                    else:
                        nc.vector.tensor_copy(vt[:], ps_p[:])

                # V natural layout via PE transpose; contiguous copies drop
                # head halves into V2 around the persistent ones cols
                for sub in range(4):
                    ps_t = psp.tile((128, 128), bf16, name=f"ps_t{ch}_{sub}",
                                    tag="o", bufs=2)
                    nc.tensor.transpose(
                        ps_t[:],
                        vt[:, sub * 128:(sub + 1) * 128],
                        ident_sb[:],
                    )
                    ti = ch * 4 + sub
                    nc.vector.tensor_copy(V2_sb[:, ti, 0:64], ps_t[:, 0:64])
                    nc.vector.tensor_copy(V2_sb[:, ti, 65:129],
                                          ps_t[:, 64:128])

            out_engs = [nc.sync, nc.gpsimd]

            # ---- attention block (b, qb) with pipelined tail ---------------
            def make_tail(b, qb, ctxA, ctxB, blk):
                g0 = b * S + qb * Q_W
                hold = {}

                def stash():
                    # on DVE: ScalarE is the attention pacing engine (exp)
                    cu = sb.tile((128, Q_W), bf16, name=f"cu{b}_{qb}",
                                 tag="cu", bufs=2)
                    nc.vector.tensor_copy(cu[0:64, :], ctxA[0:64, :])
                    nc.scalar.copy(cu[64:128, :], ctxB[0:64, :])
                    hold["cu"] = cu

                def recips():
                    st = sb.tile((128, Q_W), f32, name=f"st{b}_{qb}",
                                 tag="st", bufs=2)
                    # custom-DVE ops (reciprocal_approx_*) silently misread
                    # on HW when the AP base partition != 0, and can't read
                    # PSUM: bounce sums rows into a pre-memset SBUF tile and
                    # do ONE full-tile recip at base 0 (free-size-driven cost)
                    sti = stis[blk % 2]
                    nc.scalar.copy(sti[64:65, :], ctxA[64:65, :])
                    nc.scalar.copy(sti[0:1, :], ctxB[64:65, :])
                    nc.vector.reciprocal_approx_fast(st[:], sti[:])
                    stb = sb.tile((128, Q_W), bf16, name=f"stb{b}_{qb}",
                                  tag="stb", bufs=2)
                    nc.vector.tensor_copy(stb[:], st[:])
                    hold["stb"] = stb

                def bcast_mult():
                    stb = hold["stb"]
                    ps_bc = psp.tile((128, Q_W), f32, name=f"ps_bc{b}_{qb}",
                                     tag="o", bufs=2)
                    nc.tensor.matmul(ps_bc[0:64, :],
                                     lhsT=ones_bf[64:65, 0:64],
                                     rhs=stb[64:65, :])
                    nc.tensor.matmul(ps_bc[64:128, :],
                                     lhsT=ones_bf[0:1, 0:64],
                                     rhs=stb[0:1, :])
                    cx = sb.tile((128, Q_W), bf16, name=f"cx{b}_{qb}",
                                 tag="cx", bufs=2)
                    nc.vector.tensor_mul(cx[:], hold["cu"][:], ps_bc[:])
                    hold["cx"] = cx

                def outproj(k, last=False):
                    cx = hold["cx"]
                    for sub in (2 * k, 2 * k + 1):
                        o_sb = sb.tile((128, DM), f16,
                                       name=f"o{b}_{qb}_{sub}",
                                       tag="ob", bufs=3)
                        r0 = g0 + sub * 128
                        for nn in range(2):
                            ps_o = psp.tile((128, 512), f32,
                                            name=f"ps_o{b}_{qb}_{sub}_{nn}",
                                            tag="o", bufs=2)
                            nc.tensor.matmul(
                                ps_o[:],
                                lhsT=cx[:, sub * 128:(sub + 1) * 128],
                                rhs=wo_sb[:, nn * 512:(nn + 1) * 512],
                            )
                            # final block: evac split ACT/DVE and DMA out in
                            # 512-col halves over rotating queues -- this
                            # chain is the only unoverlapped tail
                            if last and nn == 1:
                                nc.scalar.copy(
                                    o_sb[:, 512:1024], ps_o[:])
                            else:
                                nc.vector.tensor_copy(
                                    o_sb[:, nn * 512:(nn + 1) * 512],
                                    ps_o[:])
                            if last:
                                out_engs[(sub * 2 + nn) % 2].dma_start(
                                    out[r0:r0 + 128,
                                        nn * 512:(nn + 1) * 512],
                                    o_sb[:, nn * 512:(nn + 1) * 512])
                        if not last:
                            out_engs[(blk * 4 + sub) % 2].dma_start(
                                out[r0:r0 + 128, :], o_sb[:])

                last = blk == B * NQB - 1
                if blk in DEFER:
                    # subs 2-3 deferred into the late exp-bound stretch: they
                    # are the only PE fill left once QKV chunks are exhausted
                    # (idle bubbles there re-throttle HAM to 1.2 GHz)
                    step4 = lambda: deferred.append(hold["cx"])
                else:
                    step4 = lambda: outproj(1, last)
                return [stash, recips, bcast_mult,
                        lambda: outproj(0, last), step4]

            def emit_deferred(cx, b, qb, blk, sub):
                g0 = b * S + qb * Q_W
                o_sb = sb.tile((128, DM), f16, name=f"o{b}_{qb}_{sub}",
                               tag="ob", bufs=3)
                for nn in range(2):
                    ps_o = psp.tile((128, 512), f32,
                                    name=f"ps_o{b}_{qb}_{sub}_{nn}",
                                    tag="o", bufs=2)
                    nc.tensor.matmul(
                        ps_o[:],
                        lhsT=cx[:, sub * 128:(sub + 1) * 128],
                        rhs=wo_sb[:, nn * 512:(nn + 1) * 512],
                    )
                    nc.vector.tensor_copy(
                        o_sb[:, nn * 512:(nn + 1) * 512], ps_o[:])
                r0 = g0 + sub * 128
                out_engs[(blk * 4 + sub) % 2].dma_start(
                    out[r0:r0 + 128, :], o_sb[:])

            def emit_attn(b, qb, pend, blk, fills=()):
                fills = list(fills)
                qb0 = qb * Q_W
                g0 = b * S + qb0
                n_t = (qb0 + Q_W) // 128  # causal: k-tiles needed
                ctxA = psp.tile((128, Q_W), f32, name=f"ctxA{b}_{qb}",
                                tag="ctx", bufs=2)
                ctxB = psp.tile((128, Q_W), f32, name=f"ctxB{b}_{qb}",
                                tag="ctx", bufs=2)

                avq = []  # delayed AV args: (exp_sb, t, off, w)

                def emit_av(exp_sb, t, off, w):
                    first = t == 0
                    last = t == n_t - 1
                    kti = (b * S + 128 * t) // 128
                    nc.tensor.matmul(
                        ctxA[0:65, off:Q_W],
                        lhsT=V2_sb[:, kti, 0:65],
                        rhs=exp_sb[:, 0:w],
                        start=first,
                        stop=last,
                    )
                    nc.tensor.matmul(
                        ctxB[0:65, off:Q_W],
                        lhsT=V2_sb[:, kti, 65:130],
                        rhs=exp_sb[:, 512:512 + w],
                        start=first,
                        stop=last,
                        skip_group_check=True,
                    )

                for t in range(n_t):
                    k0 = 128 * t
                    off = max(0, k0 - qb0)
                    w = Q_W - off
                    diag = k0 >= qb0
                    ps_s = psp.tile((128, 1024), f32,
                                    name=f"ps_s{b}_{qb}_{t}",
                                    tag="s", bufs=2)
                    exp_sb = sb.tile((128, 1024), bf16,
                                     name=f"exp{b}_{qb}_{t}",
                                     tag="exp", bufs=6)
                    for h in range(2):
                        nc.tensor.matmul(
                            ps_s[:, h * 512:h * 512 + w],
                            lhsT=KT_sb[h * 64:(h + 1) * 64,
                                       b * S + k0:b * S + k0 + 128],
                            rhs=QT_sb[h * 64:(h + 1) * 64,
                                      g0 + off:g0 + Q_W],
                            start=True,
                            stop=True,
                            tile_position=(h * 64, 0),
                            skip_group_check=True,
                        )
                    if t == 0 and pend:
                        pend[0]()  # stash(prev): frees ctx ring slots
                    if off == 0:
                        nc.scalar.activation(exp_sb[:, :1024], ps_s[:, :1024],
                                             Exp, scale=0.125)
                    else:
                        s3 = ps_s[:].rearrange("p (h q) -> p h q",
                                               h=2)[:, :, 0:w]
                        e3 = exp_sb[:].rearrange("p (h q) -> p h q",
                                                 h=2)[:, :, 0:w]
                        nc.scalar.activation(e3, s3, Exp, scale=0.125)
                    if t == 0 and pend:
                        pend[1]()  # recips(prev): last ctx(prev) readers
                    if diag:
                        em = exp_sb[:].rearrange("p (h q) -> p h q",
                                                 h=2)[:, :, 0:128]
                        nc.gpsimd.affine_select(
                            em, em,
                            pattern=[[0, 2], [1, 128]],
                            compare_op=is_ge,
                            fill=0.0,
                            base=0,
                            channel_multiplier=-1,
                        )
                    if t >= 1:
                        emit_av(*avq[t - 1])
                    avq.append((exp_sb, t, off, w))
                    # tail steps get slightly later priority so their PE work
                    # (bcast/outproj matmuls) can't preempt mid scores-pair
                    if t == 1 and pend:
                        with tc.high_priority(offset=-25):
                            pend[2]()  # bcast+mult(prev)
                    if t == 2 and pend:
                        with tc.high_priority(offset=-25):
                            pend[3]()  # outproj(prev) subs 0-1
                    if t == 3 and pend:
                        with tc.high_priority(offset=-25):
                            pend[4]()  # outproj(prev) subs 2-3
                    if t >= 4 and fills:
                        with tc.high_priority(offset=-25):
                            fills.pop(0)()
                emit_av(*avq[n_t - 1])
                return make_tail(b, qb, ctxA, ctxB, blk)

            # ---- interleaved emission --------------------------------------
            # QKV chunks interleave between attention blocks; chunks >= 2 get
            # a LATER scheduler priority (+offset) so they only fill true PE
            # bubbles (exp waits) instead of splitting the row-packed scores
            # pairs mid-flight (observed: 21/80 pairs broken -> ~12us lost)
            done_ch = 0
            pend = []
            DEFER = set()
            deferred = []
            # block order: batch 1 rotated so the final block is the small
            # 4-k-tile (1,0) -- its tail (the only unoverlapped one) and its
            # exp stretch are the shortest. `need` = chunks the block reads
            # (its queries + all its causal keys); chunks are emitted at the
            # correctness-minimum point and bumped later in priority, so they
            # fill exp-wait PE bubbles instead of racing attention.
            blocks = [(0, 0, 1), (0, 1, 2), (0, 2, 3), (0, 3, 4),
                      (1, 1, 6), (1, 2, 7), (1, 3, 8), (1, 0, 8)]
            for blk, (b, qb, need) in enumerate(blocks):
                while done_ch < need:
                    if done_ch >= 1:
                        with tc.high_priority(offset=-250):
                            emit_qkv(done_ch)
                    else:
                        emit_qkv(done_ch)
                    done_ch += 1
                pend = emit_attn(b, qb, pend, blk)
            for step in pend:
                step()

        if repeat == 1:
            body()
        else:
            with tc.For_i(0, repeat, 1):
                body()

    nc.compile()
    return nc


def _prep_inputs(x, Wq, bq, Wk, bk, Wv, bv, Wo):
    """Build the 8 per-core input maps (host-side sharding)."""
    x = np.asarray(x, dtype=np.float32)
    xT = x.reshape(BS, DM).T.astype(BF16)  # (DM, BS)
    # (chunk, 128, feat-tile, 512): xTc[ch, p, t, q] = xT[t*128+p, ch*512+q]
    xTc = np.ascontiguousarray(
        xT.reshape(KT_FEAT, 128, N_CH, Q_W).transpose(2, 1, 0, 3))

    def _w(W, sl):  # (128, feat-tile, CPC): w[p, t, m] = W[sl][m, t*128+p]
        wT = np.asarray(W, np.float32)[sl, :].T.astype(BF16)  # (DM, CPC)
        return np.ascontiguousarray(
            wT.reshape(KT_FEAT, 128, CPC).transpose(1, 0, 2))

    in_maps = []
    for c in range(NCORES):
        sl = slice(c * CPC, (c + 1) * CPC)
        in_maps.append({
            "xT": xTc,
            "wq": _w(Wq, sl),
            "wk": _w(Wk, sl),
            "wv": _w(Wv, sl),
            "wo": np.ascontiguousarray(np.asarray(Wo, np.float32)[:, sl].T).astype(BF16),
            "bq": np.asarray(bq, np.float32)[sl].reshape(CPC, 1).copy(),
        })
    return in_maps


def _run(in_maps, trace=False):
    global LAST_EXEC_NS, LAST_RESULTS
    from concourse import bass_utils

    if "nc" not in _CACHE:
        _CACHE["nc"] = _build()
    nc = _CACHE["nc"]
    res = bass_utils.run_bass_kernel_spmd(
        nc, in_maps, core_ids=list(range(NCORES)), trace=trace,
    )
    LAST_EXEC_NS = getattr(res, "exec_time_ns", None)
    LAST_RESULTS = res
    return res.results


def kernel(x, Wq, bq, Wk, bk, Wv, bv, Wo, bo, attention_mask=None, _trace=False):
    """Full inputs in, full output out. attention_mask is the reference's
    causal mask; causality is hardcoded in the kernel."""
    in_maps = _prep_inputs(x, Wq, bq, Wk, bk, Wv, bv, Wo)
    results = _run(in_maps, trace=_trace)
    acc = np.zeros((BS, DM), dtype=np.float32)
    for c in range(NCORES):
        acc += results[c]["out"].astype(np.float32)
    # bias folds: K bias is softmax-invariant (dropped); V bias passes through
    # attention unchanged (weights sum to 1) so it lands here via Wo
    bo_eff = (np.asarray(bo, np.float32)
              + np.asarray(bv, np.float32) @ np.asarray(Wo, np.float32).T)
    acc += bo_eff[None, :]
    return acc.reshape(B, S, DM)


# revision 60
# speedup vs baseline: 1.1741x; 1.1741x over previous
"""Multi-head attention (B=2, S=2048, DM=1024, H=16, DH=64, causal) on 8 TRN2 cores.

Sharding: tensor-parallel over heads. Core c owns heads {2c, 2c+1} = q/k/v dims
[128c, 128c+128). Each core computes its QKV projections, causal attention for
its 2 heads (both batches), and a partial output projection (row-parallel over
Wo). Host unshards by summing the 8 partials and adding bo (the TP all-reduce).

Design (v3 — AV sum fusion + full-mode AV):
  - xT DMA'd in 8 seq-chunks over 4 queues so QKV compute starts after ~0.5MB.
  - QKV chunks interleaved with attention blocks.
  - Scores: 2 heads row-packed on the PE (K=64 each, concurrent), f32 PSUM.
  - V stored as V2a=[Vh0|ones], V2b=[Vh1|ones] (128 seq, 65): each head's AV
    matmul is M=65 full 128x128 mode (no col tiling -> LDWEIGHTS can overlap
    via the background weight buffer) and row 64 accumulates the softmax
    denominator for free. No separate sum matmul.
  - Reciprocal via DVE reciprocal_approx_fast, bounced through a pre-memset
    f32 tile (custom DVE ops need base partition 0 and can't read PSUM).
  - rec broadcast across partitions via two K=1 PE matmuls in disjoint
    row+col groups (concurrent).
  - Diagonal causal mask via GpSimd affine_select on the exp tile.
  - Tail (stash/recip/bcast/mult/outproj) software-pipelined into the NEXT
    attention block's emission so no engine drains.

Causality hardcoded (the reference's attention_mask is always triu causal).
"""

import os
import sys

import numpy as np

try:
    import concourse  # noqa: F401
except ImportError:
    sys.path.insert(0, "/opt/trn_rl_repo")

import ml_dtypes

BF16 = ml_dtypes.bfloat16

B, S, DM = 2, 2048, 1024
H, DH = 16, 64
NCORES = 8
CPC = DM // NCORES  # 128 q/k/v dims per core (2 heads)
BS = B * S  # 4096
Q_W = 512  # query-block width
N_CH = BS // Q_W  # 8 seq chunks
KT_FEAT = DM // 128  # 8 contraction tiles for QKV
NQB = S // Q_W  # 4 query blocks per batch

_CACHE = {}
LAST_EXEC_NS = None
LAST_RESULTS = None


def _build(repeat=1):
    import concourse.mybir as mybir
    from concourse import bacc
    from concourse import tile
    from concourse.masks import make_identity

    f32 = mybir.dt.float32
    f16 = mybir.dt.float16
    bf16 = mybir.dt.bfloat16
    Exp = mybir.ActivationFunctionType.Exp
    Ident = mybir.ActivationFunctionType.Identity
    is_ge = mybir.AluOpType.is_ge

    nc = bacc.Bacc(
        "TRN2",
        target_bir_lowering=False,
        debug=False,
        enable_asserts=False,
        num_devices=NCORES,
    )

    # host supplies xT pre-chunked contiguous: (chunk, 128, feat-tile, 512)
    xT = nc.dram_tensor("xT", (N_CH, 128, KT_FEAT, Q_W), bf16,
                        kind="ExternalInput").ap()
    # weights pre-rearranged host-side to (128, feat-tile, CPC) contiguous
    wq = nc.dram_tensor("wq", (128, KT_FEAT, CPC), bf16,
                        kind="ExternalInput").ap()
    wk = nc.dram_tensor("wk", (128, KT_FEAT, CPC), bf16,
                        kind="ExternalInput").ap()
    wv = nc.dram_tensor("wv", (128, KT_FEAT, CPC), bf16,
                        kind="ExternalInput").ap()
    wo = nc.dram_tensor("wo", (CPC, DM), bf16, kind="ExternalInput").ap()
    bq = nc.dram_tensor("bq", (CPC, 1), f32, kind="ExternalInput").ap()
    out = nc.dram_tensor("out", (BS, DM), f16, kind="ExternalOutput").ap()

    with tile.TileContext(nc) as tc:
      with tc.tile_pool(name="consts", bufs=1) as consts, \
           tc.tile_pool(name="sb", bufs=2) as sb, \
           tc.tile_pool(name="psp", bufs=1, space="PSUM") as psp:

        def body():
            # ---- persistent tiles ------------------------------------------
            wq_sb = consts.tile((128, KT_FEAT, CPC), bf16, name="wq_sb")
            wk_sb = consts.tile((128, KT_FEAT, CPC), bf16, name="wk_sb")
            wv_sb = consts.tile((128, KT_FEAT, CPC), bf16, name="wv_sb")
            wo_sb = consts.tile((CPC, DM), bf16, name="wo_sb")
            bq_sb = consts.tile((CPC, 1), f32, name="bq_sb")

            ident_sb = consts.tile((128, 128), bf16, name="ident_sb")
            make_identity(nc, ident_sb[:])
            ones_bf = consts.tile((128, 64), bf16, name="ones_bf")
            nc.vector.memset(ones_bf[:], 1.0)

            QT_sb = consts.tile((128, BS), bf16, name="QT_sb")
            KT_sb = consts.tile((128, BS), bf16, name="KT_sb")
            # V2: (seq 128, 32 k-tiles, 130) = [Vh0 d0-63 | ones | Vh1 | ones]
            # -> AV lhsT [0:65] / [65:130] gives each head [V|ones] (M=65,
            # full-mode matmul whose row 64 accumulates the softmax denom).
            V2_sb = consts.tile((128, BS // 128, 130), bf16, name="V2_sb")
            nc.vector.memset(V2_sb[:, :, 64:65], 1.0)
            nc.vector.memset(V2_sb[:, :, 129:130], 1.0)

            # xT in 8 per-chunk tiles so QKV(ch) only waits its own slab.
            # Startup: wq halves land on two queues while chunk 0 streams in
            # quarters over 4 queues -> first Q-proj matmul fires ~5us in.
            xts = [consts.tile((128, KT_FEAT, Q_W), bf16, name=f"xt{ch}")
                   for ch in range(N_CH)]
            # startup DMA: wq / chunk-0 / wk / wv slabs interleaved across the
            # three queues in PE consumption order, so the first Q-proj
            # matmul fires as soon as wq+first slab land (~4us)
            nc.sync.dma_start(bq_sb[:], bq)
            nc.scalar.dma_start(wq_sb[:, 0:3], wq[:, 0:3])
            nc.gpsimd.dma_start(wq_sb[:, 3:6], wq[:, 3:6])
            nc.sync.dma_start(wq_sb[:, 6:8], wq[:, 6:8])
            nc.sync.dma_start(xts[0][:, 0:1], xT[0, :, 0:1])
            nc.scalar.dma_start(xts[0][:, 1:2], xT[0, :, 1:2])
            nc.gpsimd.dma_start(xts[0][:, 2:3], xT[0, :, 2:3])
            nc.sync.dma_start(xts[0][:, 3:5], xT[0, :, 3:5])
            nc.scalar.dma_start(wk_sb[:, 0:3], wk[:, 0:3])
            nc.gpsimd.dma_start(wk_sb[:, 3:6], wk[:, 3:6])
            nc.sync.dma_start(wk_sb[:, 6:8], wk[:, 6:8])
            nc.scalar.dma_start(xts[0][:, 5:6], xT[0, :, 5:6])
            nc.gpsimd.dma_start(xts[0][:, 6:8], xT[0, :, 6:8])
            nc.scalar.dma_start(wv_sb[:, 0:3], wv[:, 0:3])
            nc.gpsimd.dma_start(wv_sb[:, 3:6], wv[:, 3:6])
            nc.sync.dma_start(wv_sb[:, 6:8], wv[:, 6:8])
            nc.scalar.dma_start(xts[1][:, 0:4], xT[1, :, 0:4])
            nc.gpsimd.dma_start(xts[1][:, 4:8], xT[1, :, 4:8])
            nc.sync.dma_start(wo_sb[:], wo)
            dma_engs = [nc.sync, nc.scalar, nc.gpsimd]
            for ch in range(2, N_CH):
                dma_engs[ch % 3].dma_start(xts[ch][:], xT[ch])

            # persistent pre-memset recip staging (ping-pong): full-tile
            # reciprocal_approx_fast needs every partition initialized, and
            # memset in the per-block chain would delay PSUM release
            stis = []
            for i in range(2):
                sti = consts.tile((128, Q_W), f32, name=f"sti{i}")
                nc.vector.memset(sti[:], 1.0)
                stis.append(sti)

            # ---- QKV chunk -------------------------------------------------
            def emit_qkv(ch):
                c0 = ch * Q_W
                vt = sb.tile((128, Q_W), bf16, name=f"vt{ch}", tag="vt",
                             bufs=2)
                # bias folds: scores use (Q+bq)@K ((Q+bq)@bk is constant over
                # keys -> softmax-invariant), and bv folds into bo host-side
                # (sum of attn weights is 1). Only Q needs its bias; K/V
                # evacuation becomes plain DVE copies, freeing ScalarE (the
                # attention-phase pacing engine) for exp.
                for pname, w_sb in (
                    ("q", wq_sb),
                    ("k", wk_sb),
                    ("v", wv_sb),
                ):
                    ps_p = psp.tile((128, Q_W), f32, name=f"ps_{pname}{ch}",
                                    tag="s", bufs=2)
                    for t in range(KT_FEAT):
                        nc.tensor.matmul(
                            ps_p[:],
                            lhsT=w_sb[:, t, :],
                            rhs=xts[ch][:, t, :],
                            start=(t == 0),
                            stop=(t == KT_FEAT - 1),
                        )
                    if pname == "q":
                        nc.scalar.activation(QT_sb[:, c0:c0 + Q_W], ps_p[:],
                                             Ident, bias=bq_sb[:])
                    elif pname == "k":
                        nc.vector.tensor_copy(KT_sb[:, c0:c0 + Q_W], ps_p[:])
                    else:
                        nc.vector.tensor_copy(vt[:], ps_p[:])

                # V natural layout via PE transpose; contiguous copies drop
                # head halves into V2 around the persistent ones cols
                for sub in range(4):
                    ps_t = psp.tile((128, 128), bf16, name=f"ps_t{ch}_{sub}",
                                    tag="o", bufs=2)
                    nc.tensor.transpose(
                        ps_t[:],
                        vt[:, sub * 128:(sub + 1) * 128],
                        ident_sb[:],
                    )
                    ti = ch * 4 + sub
                    nc.vector.tensor_copy(V2_sb[:, ti, 0:64], ps_t[:, 0:64])
                    nc.vector.tensor_copy(V2_sb[:, ti, 65:129],
                                          ps_t[:, 64:128])

            out_engs = [nc.sync, nc.scalar, nc.gpsimd]

            # ---- attention block (b, qb) with pipelined tail ---------------
            def make_tail(b, qb, ctxA, ctxB, blk):
                g0 = b * S + qb * Q_W
                hold = {}

                def stash():
                    # on DVE: ScalarE is the attention pacing engine (exp)
                    cu = sb.tile((128, Q_W), bf16, name=f"cu{b}_{qb}",
                                 tag="cu", bufs=2)
                    nc.vector.tensor_copy(cu[0:64, :], ctxA[0:64, :])
                    nc.scalar.copy(cu[64:128, :], ctxB[0:64, :])
                    hold["cu"] = cu

                def recips():
                    st = sb.tile((128, Q_W), f32, name=f"st{b}_{qb}",
                                 tag="st", bufs=2)
                    # custom-DVE ops (reciprocal_approx_*) silently misread
                    # on HW when the AP base partition != 0, and can't read
                    # PSUM: bounce sums rows into a pre-memset SBUF tile and
                    # do ONE full-tile recip at base 0 (free-size-driven cost)
                    sti = stis[blk % 2]
                    nc.scalar.copy(sti[64:65, :], ctxA[64:65, :])
                    nc.scalar.copy(sti[0:1, :], ctxB[64:65, :])
                    nc.vector.reciprocal_approx_fast(st[:], sti[:])
                    stb = sb.tile((128, Q_W), bf16, name=f"stb{b}_{qb}",
                                  tag="stb", bufs=2)
                    nc.vector.tensor_copy(stb[:], st[:])
                    hold["stb"] = stb

                def bcast_mult():
                    stb = hold["stb"]
                    ps_bc = psp.tile((128, Q_W), f32, name=f"ps_bc{b}_{qb}",
                                     tag="o", bufs=2)
                    nc.tensor.matmul(ps_bc[0:64, :],
                                     lhsT=ones_bf[64:65, 0:64],
                                     rhs=stb[64:65, :])
                    nc.tensor.matmul(ps_bc[64:128, :],
                                     lhsT=ones_bf[0:1, 0:64],
                                     rhs=stb[0:1, :])
                    cx = sb.tile((128, Q_W), bf16, name=f"cx{b}_{qb}",
                                 tag="cx", bufs=2)
                    nc.vector.tensor_mul(cx[:], hold["cu"][:], ps_bc[:])
                    hold["cx"] = cx

                def outproj(k, last=False):
                    cx = hold["cx"]
                    for sub in (2 * k, 2 * k + 1):
                        o_sb = sb.tile((128, DM), f16,
                                       name=f"o{b}_{qb}_{sub}",
                                       tag="ob", bufs=3)
                        r0 = g0 + sub * 128
                        for nn in range(2):
                            ps_o = psp.tile((128, 512), f32,
                                            name=f"ps_o{b}_{qb}_{sub}_{nn}",
                                            tag="o", bufs=2)
                            nc.tensor.matmul(
                                ps_o[:],
                                lhsT=cx[:, sub * 128:(sub + 1) * 128],
                                rhs=wo_sb[:, nn * 512:(nn + 1) * 512],
                            )
                            # final block: evac split ACT/DVE and DMA out in
                            # 512-col halves over rotating queues -- this
                            # chain is the only unoverlapped tail
                            if last and nn == 1:
                                nc.scalar.copy(
                                    o_sb[:, 512:1024], ps_o[:])
                            else:
                                nc.vector.tensor_copy(
                                    o_sb[:, nn * 512:(nn + 1) * 512],
                                    ps_o[:])
                            if last:
                                out_engs[(sub * 2 + nn) % 3].dma_start(
                                    out[r0:r0 + 128,
                                        nn * 512:(nn + 1) * 512],
                                    o_sb[:, nn * 512:(nn + 1) * 512])
                        if not last:
                            out_engs[(blk * 4 + sub) % 3].dma_start(
                                out[r0:r0 + 128, :], o_sb[:])

                last = blk == B * NQB - 1
                if blk in DEFER:
                    # subs 2-3 deferred into the late exp-bound stretch: they
                    # are the only PE fill left once QKV chunks are exhausted
                    # (idle bubbles there re-throttle HAM to 1.2 GHz)
                    step4 = lambda: deferred.append(hold["cx"])
                else:
                    step4 = lambda: outproj(1, last)
                return [stash, recips, bcast_mult,
                        lambda: outproj(0, last), step4]

            def emit_deferred(cx, b, qb, blk, sub):
                g0 = b * S + qb * Q_W
                o_sb = sb.tile((128, DM), f16, name=f"o{b}_{qb}_{sub}",
                               tag="ob", bufs=3)
                for nn in range(2):
                    ps_o = psp.tile((128, 512), f32,
                                    name=f"ps_o{b}_{qb}_{sub}_{nn}",
                                    tag="o", bufs=2)
                    nc.tensor.matmul(
                        ps_o[:],
                        lhsT=cx[:, sub * 128:(sub + 1) * 128],
                        rhs=wo_sb[:, nn * 512:(nn + 1) * 512],
                    )
                    nc.vector.tensor_copy(
                        o_sb[:, nn * 512:(nn + 1) * 512], ps_o[:])
                r0 = g0 + sub * 128
                out_engs[(blk * 4 + sub) % 3].dma_start(
                    out[r0:r0 + 128, :], o_sb[:])

            def emit_attn(b, qb, pend, blk, fills=()):
                fills = list(fills)
                qb0 = qb * Q_W
                g0 = b * S + qb0
                n_t = (qb0 + Q_W) // 128  # causal: k-tiles needed
                ctxA = psp.tile((128, Q_W), f32, name=f"ctxA{b}_{qb}",
                                tag="ctx", bufs=2)
                ctxB = psp.tile((128, Q_W), f32, name=f"ctxB{b}_{qb}",
                                tag="ctx", bufs=2)

                avq = []  # delayed AV args: (exp_sb, t, off, w)

                def emit_av(exp_sb, t, off, w):
                    first = t == 0
                    last = t == n_t - 1
                    kti = (b * S + 128 * t) // 128
                    nc.tensor.matmul(
                        ctxA[0:65, off:Q_W],
                        lhsT=V2_sb[:, kti, 0:65],
                        rhs=exp_sb[:, 0:w],
                        start=first,
                        stop=last,
                    )
                    nc.tensor.matmul(
                        ctxB[0:65, off:Q_W],
                        lhsT=V2_sb[:, kti, 65:130],
                        rhs=exp_sb[:, 512:512 + w],
                        start=first,
                        stop=last,
                        skip_group_check=True,
                    )

                for t in range(n_t):
                    k0 = 128 * t
                    off = max(0, k0 - qb0)
                    w = Q_W - off
                    diag = k0 >= qb0
                    ps_s = psp.tile((128, 1024), f32,
                                    name=f"ps_s{b}_{qb}_{t}",
                                    tag="s", bufs=2)
                    exp_sb = sb.tile((128, 1024), bf16,
                                     name=f"exp{b}_{qb}_{t}",
                                     tag="exp", bufs=6)
                    for h in range(2):
                        nc.tensor.matmul(
                            ps_s[:, h * 512:h * 512 + w],
                            lhsT=KT_sb[h * 64:(h + 1) * 64,
                                       b * S + k0:b * S + k0 + 128],
                            rhs=QT_sb[h * 64:(h + 1) * 64,
                                      g0 + off:g0 + Q_W],
                            start=True,
                            stop=True,
                            tile_position=(h * 64, 0),
                            skip_group_check=True,
                        )
                    if t == 0 and pend:
                        pend[0]()  # stash(prev): frees ctx ring slots
                    if off == 0:
                        nc.scalar.activation(exp_sb[:, :1024], ps_s[:, :1024],
                                             Exp, scale=0.125)
                    else:
                        s3 = ps_s[:].rearrange("p (h q) -> p h q",
                                               h=2)[:, :, 0:w]
                        e3 = exp_sb[:].rearrange("p (h q) -> p h q",
                                                 h=2)[:, :, 0:w]
                        nc.scalar.activation(e3, s3, Exp, scale=0.125)
                    if t == 0 and pend:
                        pend[1]()  # recips(prev): last ctx(prev) readers
                    if diag:
                        em = exp_sb[:].rearrange("p (h q) -> p h q",
                                                 h=2)[:, :, 0:128]
                        nc.gpsimd.affine_select(
                            em, em,
                            pattern=[[0, 2], [1, 128]],
                            compare_op=is_ge,
                            fill=0.0,
                            base=0,
                            channel_multiplier=-1,
                        )
                    if t >= 1:
                        emit_av(*avq[t - 1])
                    avq.append((exp_sb, t, off, w))
                    # tail steps get slightly later priority so their PE work
                    # (bcast/outproj matmuls) can't preempt mid scores-pair
                    if t == 1 and pend:
                        with tc.high_priority(offset=-25):
                            pend[2]()  # bcast+mult(prev)
                    if t == 2 and pend:
                        with tc.high_priority(offset=-25):
                            pend[3]()  # outproj(prev) subs 0-1
                    if t == 3 and pend:
                        with tc.high_priority(offset=-25):
                            pend[4]()  # outproj(prev) subs 2-3
                    if t >= 4 and fills:
                        with tc.high_priority(offset=-25):
                            fills.pop(0)()
                emit_av(*avq[n_t - 1])
                return make_tail(b, qb, ctxA, ctxB, blk)

            # ---- interleaved emission --------------------------------------
            # QKV chunks interleave between attention blocks; chunks >= 2 get
            # a LATER scheduler priority (+offset) so they only fill true PE
            # bubbles (exp waits) instead of splitting the row-packed scores
            # pairs mid-flight (observed: 21/80 pairs broken -> ~12us lost)
            done_ch = 0
            pend = []
            DEFER = set()
            deferred = []
            # block order: batch 1 rotated so the final block is the small
            # 4-k-tile (1,0) -- its tail (the only unoverlapped one) and its
            # exp stretch are the shortest. `need` = chunks the block reads
            # (its queries + all its causal keys); chunks are emitted at the
            # correctness-minimum point and bumped later in priority, so they
            # fill exp-wait PE bubbles instead of racing attention.
            blocks = [(0, 0, 1), (0, 1, 2), (0, 2, 3), (0, 3, 4),
                      (1, 1, 6), (1, 2, 7), (1, 3, 8), (1, 0, 8)]
            for blk, (b, qb, need) in enumerate(blocks):
                while done_ch < need:
                    if done_ch >= 1:
                        with tc.high_priority(offset=-250):
                            emit_qkv(done_ch)
                    else:
                        emit_qkv(done_ch)
                    done_ch += 1
                pend = emit_attn(b, qb, pend, blk)
            for step in pend:
                step()

        if repeat == 1:
            body()
        else:
            with tc.For_i(0, repeat, 1):
                body()

    nc.compile()
    return nc


def _prep_inputs(x, Wq, bq, Wk, bk, Wv, bv, Wo):
    """Build the 8 per-core input maps (host-side sharding)."""
    x = np.asarray(x, dtype=np.float32)
    xT = x.reshape(BS, DM).T.astype(BF16)  # (DM, BS)
    # (chunk, 128, feat-tile, 512): xTc[ch, p, t, q] = xT[t*128+p, ch*512+q]
    xTc = np.ascontiguousarray(
        xT.reshape(KT_FEAT, 128, N_CH, Q_W).transpose(2, 1, 0, 3))

    def _w(W, sl):  # (128, feat-tile, CPC): w[p, t, m] = W[sl][m, t*128+p]
        wT = np.asarray(W, np.float32)[sl, :].T.astype(BF16)  # (DM, CPC)
        return np.ascontiguousarray(
            wT.reshape(KT_FEAT, 128, CPC).transpose(1, 0, 2))

    in_maps = []
    for c in range(NCORES):
        sl = slice(c * CPC, (c + 1) * CPC)
        in_maps.append({
            "xT": xTc,
            "wq": _w(Wq, sl),
            "wk": _w(Wk, sl),
            "wv": _w(Wv, sl),
            "wo": np.ascontiguousarray(np.asarray(Wo, np.float32)[:, sl].T).astype(BF16),
            "bq": np.asarray(bq, np.float32)[sl].reshape(CPC, 1).copy(),
        })
    return in_maps


def _run(in_maps, trace=False):
    global LAST_EXEC_NS, LAST_RESULTS
    from concourse import bass_utils

    if "nc" not in _CACHE:
        _CACHE["nc"] = _build()
    nc = _CACHE["nc"]
    res = bass_utils.run_bass_kernel_spmd(
        nc, in_maps, core_ids=list(range(NCORES)), trace=trace,
    )
    LAST_EXEC_NS = getattr(res, "exec_time_ns", None)
    LAST_RESULTS = res
    return res.results


def kernel(x, Wq, bq, Wk, bk, Wv, bv, Wo, bo, attention_mask=None, _trace=False):
    """Full inputs in, full output out. attention_mask is the reference's
    causal mask; causality is hardcoded in the kernel."""
    in_maps = _prep_inputs(x, Wq, bq, Wk, bk, Wv, bv, Wo)
    results = _run(in_maps, trace=_trace)
    acc = np.zeros((BS, DM), dtype=np.float32)
    for c in range(NCORES):
        acc += results[c]["out"].astype(np.float32)
    # bias folds: K bias is softmax-invariant (dropped); V bias passes through
    # attention unchanged (weights sum to 1) so it lands here via Wo
    bo_eff = (np.asarray(bo, np.float32)
              + np.asarray(bv, np.float32) @ np.asarray(Wo, np.float32).T)
    acc += bo_eff[None, :]
    return acc.reshape(B, S, DM)


# revision 61
# speedup vs baseline: 1.1759x; 1.0015x over previous
"""Multi-head attention (B=2, S=2048, DM=1024, H=16, DH=64, causal) on 8 TRN2 cores.

Sharding: tensor-parallel over heads. Core c owns heads {2c, 2c+1} = q/k/v dims
[128c, 128c+128). Each core computes its QKV projections, causal attention for
its 2 heads (both batches), and a partial output projection (row-parallel over
Wo). Host unshards by summing the 8 partials and adding bo (the TP all-reduce).

Design (v3 — AV sum fusion + full-mode AV):
  - xT DMA'd in 8 seq-chunks over 4 queues so QKV compute starts after ~0.5MB.
  - QKV chunks interleaved with attention blocks.
  - Scores: 2 heads row-packed on the PE (K=64 each, concurrent), f32 PSUM.
  - V stored as V2a=[Vh0|ones], V2b=[Vh1|ones] (128 seq, 65): each head's AV
    matmul is M=65 full 128x128 mode (no col tiling -> LDWEIGHTS can overlap
    via the background weight buffer) and row 64 accumulates the softmax
    denominator for free. No separate sum matmul.
  - Reciprocal via DVE reciprocal_approx_fast, bounced through a pre-memset
    f32 tile (custom DVE ops need base partition 0 and can't read PSUM).
  - rec broadcast across partitions via two K=1 PE matmuls in disjoint
    row+col groups (concurrent).
  - Diagonal causal mask via GpSimd affine_select on the exp tile.
  - Tail (stash/recip/bcast/mult/outproj) software-pipelined into the NEXT
    attention block's emission so no engine drains.

Causality hardcoded (the reference's attention_mask is always triu causal).
"""

import os
import sys

import numpy as np

try:
    import concourse  # noqa: F401
except ImportError:
    sys.path.insert(0, "/opt/trn_rl_repo")

import ml_dtypes

BF16 = ml_dtypes.bfloat16

B, S, DM = 2, 2048, 1024
H, DH = 16, 64
NCORES = 8
CPC = DM // NCORES  # 128 q/k/v dims per core (2 heads)
BS = B * S  # 4096
Q_W = 512  # query-block width
N_CH = BS // Q_W  # 8 seq chunks
KT_FEAT = DM // 128  # 8 contraction tiles for QKV
NQB = S // Q_W  # 4 query blocks per batch

_CACHE = {}
LAST_EXEC_NS = None
LAST_RESULTS = None


def _build(repeat=1):
    import concourse.mybir as mybir
    from concourse import bacc
    from concourse import tile
    from concourse.masks import make_identity

    f32 = mybir.dt.float32
    f16 = mybir.dt.float16
    bf16 = mybir.dt.bfloat16
    Exp = mybir.ActivationFunctionType.Exp
    Ident = mybir.ActivationFunctionType.Identity
    is_ge = mybir.AluOpType.is_ge

    nc = bacc.Bacc(
        "TRN2",
        target_bir_lowering=False,
        debug=False,
        enable_asserts=False,
        num_devices=NCORES,
    )

    # host supplies xT pre-chunked contiguous: (chunk, 128, feat-tile, 512)
    xT = nc.dram_tensor("xT", (N_CH, 128, KT_FEAT, Q_W), bf16,
                        kind="ExternalInput").ap()
    # weights pre-rearranged host-side to (128, feat-tile, CPC) contiguous
    wq = nc.dram_tensor("wq", (128, KT_FEAT, CPC), bf16,
                        kind="ExternalInput").ap()
    wk = nc.dram_tensor("wk", (128, KT_FEAT, CPC), bf16,
                        kind="ExternalInput").ap()
    wv = nc.dram_tensor("wv", (128, KT_FEAT, CPC), bf16,
                        kind="ExternalInput").ap()
    wo = nc.dram_tensor("wo", (CPC, DM), bf16, kind="ExternalInput").ap()
    bq = nc.dram_tensor("bq", (CPC, 1), f32, kind="ExternalInput").ap()
    out = nc.dram_tensor("out", (BS, DM), f16, kind="ExternalOutput").ap()

    with tile.TileContext(nc) as tc:
      with tc.tile_pool(name="consts", bufs=1) as consts, \
           tc.tile_pool(name="sb", bufs=2) as sb, \
           tc.tile_pool(name="psp", bufs=1, space="PSUM") as psp:

        def body():
            # ---- persistent tiles ------------------------------------------
            wq_sb = consts.tile((128, KT_FEAT, CPC), bf16, name="wq_sb")
            wk_sb = consts.tile((128, KT_FEAT, CPC), bf16, name="wk_sb")
            wv_sb = consts.tile((128, KT_FEAT, CPC), bf16, name="wv_sb")
            wo_sb = consts.tile((CPC, DM), bf16, name="wo_sb")
            bq_sb = consts.tile((CPC, 1), f32, name="bq_sb")

            ident_sb = consts.tile((128, 128), bf16, name="ident_sb")
            make_identity(nc, ident_sb[:])
            ones_bf = consts.tile((128, 64), bf16, name="ones_bf")
            nc.vector.memset(ones_bf[:], 1.0)

            QT_sb = consts.tile((128, BS), bf16, name="QT_sb")
            KT_sb = consts.tile((128, BS), bf16, name="KT_sb")
            # V2: (seq 128, 32 k-tiles, 130) = [Vh0 d0-63 | ones | Vh1 | ones]
            # -> AV lhsT [0:65] / [65:130] gives each head [V|ones] (M=65,
            # full-mode matmul whose row 64 accumulates the softmax denom).
            V2_sb = consts.tile((128, BS // 128, 130), bf16, name="V2_sb")
            nc.vector.memset(V2_sb[:, :, 64:65], 1.0)
            nc.vector.memset(V2_sb[:, :, 129:130], 1.0)

            # xT in 8 per-chunk tiles so QKV(ch) only waits its own slab.
            # Startup: wq halves land on two queues while chunk 0 streams in
            # quarters over 4 queues -> first Q-proj matmul fires ~5us in.
            xts = [consts.tile((128, KT_FEAT, Q_W), bf16, name=f"xt{ch}")
                   for ch in range(N_CH)]
            # startup DMA: wq / chunk-0 / wk / wv slabs interleaved across the
            # three queues in PE consumption order, so the first Q-proj
            # matmul fires as soon as wq+first slab land (~4us)
            nc.sync.dma_start(bq_sb[:], bq)
            nc.scalar.dma_start(wq_sb[:, 0:3], wq[:, 0:3])
            nc.gpsimd.dma_start(wq_sb[:, 3:6], wq[:, 3:6])
            nc.sync.dma_start(wq_sb[:, 6:8], wq[:, 6:8])
            nc.sync.dma_start(xts[0][:, 0:1], xT[0, :, 0:1])
            nc.scalar.dma_start(xts[0][:, 1:2], xT[0, :, 1:2])
            nc.gpsimd.dma_start(xts[0][:, 2:3], xT[0, :, 2:3])
            nc.sync.dma_start(xts[0][:, 3:5], xT[0, :, 3:5])
            nc.scalar.dma_start(wk_sb[:, 0:3], wk[:, 0:3])
            nc.gpsimd.dma_start(wk_sb[:, 3:6], wk[:, 3:6])
            nc.sync.dma_start(wk_sb[:, 6:8], wk[:, 6:8])
            nc.scalar.dma_start(xts[0][:, 5:6], xT[0, :, 5:6])
            nc.gpsimd.dma_start(xts[0][:, 6:8], xT[0, :, 6:8])
            nc.scalar.dma_start(wv_sb[:, 0:3], wv[:, 0:3])
            nc.gpsimd.dma_start(wv_sb[:, 3:6], wv[:, 3:6])
            nc.sync.dma_start(wv_sb[:, 6:8], wv[:, 6:8])
            nc.scalar.dma_start(xts[1][:, 0:4], xT[1, :, 0:4])
            nc.gpsimd.dma_start(xts[1][:, 4:8], xT[1, :, 4:8])
            nc.sync.dma_start(wo_sb[:], wo)
            dma_engs = [nc.sync, nc.scalar, nc.gpsimd]
            for ch in range(2, N_CH):
                dma_engs[ch % 3].dma_start(xts[ch][:], xT[ch])

            # persistent pre-memset recip staging (ping-pong): full-tile
            # reciprocal_approx_fast needs every partition initialized, and
            # memset in the per-block chain would delay PSUM release
            stis = []
            for i in range(2):
                sti = consts.tile((128, Q_W), f32, name=f"sti{i}")
                nc.vector.memset(sti[:], 1.0)
                stis.append(sti)

            # ---- QKV chunk -------------------------------------------------
            def emit_qkv(ch):
                c0 = ch * Q_W
                vt = sb.tile((128, Q_W), bf16, name=f"vt{ch}", tag="vt",
                             bufs=2)
                # bias folds: scores use (Q+bq)@K ((Q+bq)@bk is constant over
                # keys -> softmax-invariant), and bv folds into bo host-side
                # (sum of attn weights is 1). Only Q needs its bias; K/V
                # evacuation becomes plain DVE copies, freeing ScalarE (the
                # attention-phase pacing engine) for exp.
                for pname, w_sb in (
                    ("q", wq_sb),
                    ("k", wk_sb),
                    ("v", wv_sb),
                ):
                    ps_p = psp.tile((128, Q_W), f32, name=f"ps_{pname}{ch}",
                                    tag="s", bufs=2)
                    for t in range(KT_FEAT):
                        nc.tensor.matmul(
                            ps_p[:],
                            lhsT=w_sb[:, t, :],
                            rhs=xts[ch][:, t, :],
                            start=(t == 0),
                            stop=(t == KT_FEAT - 1),
                        )
                    if pname == "q":
                        nc.scalar.activation(QT_sb[:, c0:c0 + Q_W], ps_p[:],
                                             Ident, bias=bq_sb[:])
                    elif pname == "k":
                        nc.vector.tensor_copy(KT_sb[:, c0:c0 + Q_W], ps_p[:])
                    else:
                        nc.vector.tensor_copy(vt[:], ps_p[:])

                # V natural layout via PE transpose; contiguous copies drop
                # head halves into V2 around the persistent ones cols
                for sub in range(4):
                    ps_t = psp.tile((128, 128), bf16, name=f"ps_t{ch}_{sub}",
                                    tag="o", bufs=2)
                    nc.tensor.transpose(
                        ps_t[:],
                        vt[:, sub * 128:(sub + 1) * 128],
                        ident_sb[:],
                    )
                    ti = ch * 4 + sub
                    nc.vector.tensor_copy(V2_sb[:, ti, 0:64], ps_t[:, 0:64])
                    nc.vector.tensor_copy(V2_sb[:, ti, 65:129],
                                          ps_t[:, 64:128])

            out_engs = [nc.sync, nc.scalar, nc.gpsimd]

            # ---- attention block (b, qb) with pipelined tail ---------------
            def make_tail(b, qb, ctxA, ctxB, blk):
                g0 = b * S + qb * Q_W
                hold = {}

                def stash():
                    # on DVE: ScalarE is the attention pacing engine (exp)
                    cu = sb.tile((128, Q_W), bf16, name=f"cu{b}_{qb}",
                                 tag="cu", bufs=2)
                    nc.vector.tensor_copy(cu[0:64, :], ctxA[0:64, :])
                    nc.scalar.copy(cu[64:128, :], ctxB[0:64, :])
                    hold["cu"] = cu

                def recips():
                    st = sb.tile((128, Q_W), f32, name=f"st{b}_{qb}",
                                 tag="st", bufs=2)
                    # custom-DVE ops (reciprocal_approx_*) silently misread
                    # on HW when the AP base partition != 0, and can't read
                    # PSUM: bounce sums rows into a pre-memset SBUF tile and
                    # do ONE full-tile recip at base 0 (free-size-driven cost)
                    sti = stis[blk % 2]
                    nc.scalar.copy(sti[64:65, :], ctxA[64:65, :])
                    nc.scalar.copy(sti[0:1, :], ctxB[64:65, :])
                    nc.vector.reciprocal_approx_fast(st[:], sti[:])
                    stb = sb.tile((128, Q_W), bf16, name=f"stb{b}_{qb}",
                                  tag="stb", bufs=2)
                    nc.vector.tensor_copy(stb[:], st[:])
                    hold["stb"] = stb

                def bcast_mult():
                    stb = hold["stb"]
                    ps_bc = psp.tile((128, Q_W), f32, name=f"ps_bc{b}_{qb}",
                                     tag="o", bufs=2)
                    nc.tensor.matmul(ps_bc[0:64, :],
                                     lhsT=ones_bf[64:65, 0:64],
                                     rhs=stb[64:65, :])
                    nc.tensor.matmul(ps_bc[64:128, :],
                                     lhsT=ones_bf[0:1, 0:64],
                                     rhs=stb[0:1, :])
                    cx = sb.tile((128, Q_W), bf16, name=f"cx{b}_{qb}",
                                 tag="cx", bufs=2)
                    nc.vector.tensor_mul(cx[:], hold["cu"][:], ps_bc[:])
                    hold["cx"] = cx

                def outproj(k, last=False):
                    cx = hold["cx"]
                    for sub in (2 * k, 2 * k + 1):
                        o_sb = sb.tile((128, DM), f16,
                                       name=f"o{b}_{qb}_{sub}",
                                       tag="ob", bufs=3)
                        r0 = g0 + sub * 128
                        for nn in range(2):
                            ps_o = psp.tile((128, 512), f32,
                                            name=f"ps_o{b}_{qb}_{sub}_{nn}",
                                            tag="o", bufs=2)
                            nc.tensor.matmul(
                                ps_o[:],
                                lhsT=cx[:, sub * 128:(sub + 1) * 128],
                                rhs=wo_sb[:, nn * 512:(nn + 1) * 512],
                            )
                            # final block: evac split ACT/DVE and DMA out in
                            # 512-col halves over rotating queues -- this
                            # chain is the only unoverlapped tail
                            if last and nn == 1:
                                nc.scalar.copy(
                                    o_sb[:, 512:1024], ps_o[:])
                            else:
                                nc.vector.tensor_copy(
                                    o_sb[:, nn * 512:(nn + 1) * 512],
                                    ps_o[:])
                            if last:
                                out_engs[(sub * 2 + nn) % 3].dma_start(
                                    out[r0:r0 + 128,
                                        nn * 512:(nn + 1) * 512],
                                    o_sb[:, nn * 512:(nn + 1) * 512])
                        if not last:
                            out_engs[(blk * 4 + sub) % 3].dma_start(
                                out[r0:r0 + 128, :], o_sb[:])

                last = blk == B * NQB - 1
                return [stash, recips, bcast_mult,
                        lambda: outproj(0, last), lambda: outproj(1, last)]


            def emit_attn(b, qb, pend, blk):
                qb0 = qb * Q_W
                g0 = b * S + qb0
                n_t = (qb0 + Q_W) // 128  # causal: k-tiles needed
                ctxA = psp.tile((128, Q_W), f32, name=f"ctxA{b}_{qb}",
                                tag="ctx", bufs=2)
                ctxB = psp.tile((128, Q_W), f32, name=f"ctxB{b}_{qb}",
                                tag="ctx", bufs=2)

                avq = []  # delayed AV args: (exp_sb, t, off, w)

                def emit_av(exp_sb, t, off, w):
                    first = t == 0
                    last = t == n_t - 1
                    kti = (b * S + 128 * t) // 128
                    nc.tensor.matmul(
                        ctxA[0:65, off:Q_W],
                        lhsT=V2_sb[:, kti, 0:65],
                        rhs=exp_sb[:, 0:w],
                        start=first,
                        stop=last,
                    )
                    nc.tensor.matmul(
                        ctxB[0:65, off:Q_W],
                        lhsT=V2_sb[:, kti, 65:130],
                        rhs=exp_sb[:, 512:512 + w],
                        start=first,
                        stop=last,
                        skip_group_check=True,
                    )

                for t in range(n_t):
                    k0 = 128 * t
                    off = max(0, k0 - qb0)
                    w = Q_W - off
                    diag = k0 >= qb0
                    ps_s = psp.tile((128, 1024), f32,
                                    name=f"ps_s{b}_{qb}_{t}",
                                    tag="s", bufs=2)
                    exp_sb = sb.tile((128, 1024), bf16,
                                     name=f"exp{b}_{qb}_{t}",
                                     tag="exp", bufs=6)
                    for h in range(2):
                        nc.tensor.matmul(
                            ps_s[:, h * 512:h * 512 + w],
                            lhsT=KT_sb[h * 64:(h + 1) * 64,
                                       b * S + k0:b * S + k0 + 128],
                            rhs=QT_sb[h * 64:(h + 1) * 64,
                                      g0 + off:g0 + Q_W],
                            start=True,
                            stop=True,
                            tile_position=(h * 64, 0),
                            skip_group_check=True,
                        )
                    if t == 0 and pend:
                        pend[0]()  # stash(prev): frees ctx ring slots
                    if off == 0:
                        nc.scalar.activation(exp_sb[:, :1024], ps_s[:, :1024],
                                             Exp, scale=0.125)
                    else:
                        s3 = ps_s[:].rearrange("p (h q) -> p h q",
                                               h=2)[:, :, 0:w]
                        e3 = exp_sb[:].rearrange("p (h q) -> p h q",
                                                 h=2)[:, :, 0:w]
                        nc.scalar.activation(e3, s3, Exp, scale=0.125)
                    if t == 0 and pend:
                        pend[1]()  # recips(prev): last ctx(prev) readers
                    if diag:
                        em = exp_sb[:].rearrange("p (h q) -> p h q",
                                                 h=2)[:, :, 0:128]
                        nc.gpsimd.affine_select(
                            em, em,
                            pattern=[[0, 2], [1, 128]],
                            compare_op=is_ge,
                            fill=0.0,
                            base=0,
                            channel_multiplier=-1,
                        )
                    if t >= 1:
                        emit_av(*avq[t - 1])
                    avq.append((exp_sb, t, off, w))
                    # tail steps get slightly later priority so their PE work
                    # (bcast/outproj matmuls) can't preempt mid scores-pair
                    if t == 1 and pend:
                        with tc.high_priority(offset=-25):
                            pend[2]()  # bcast+mult(prev)
                    if t == 2 and pend:
                        with tc.high_priority(offset=-25):
                            pend[3]()  # outproj(prev) subs 0-1
                    if t == 3 and pend:
                        with tc.high_priority(offset=-25):
                            pend[4]()  # outproj(prev) subs 2-3
                emit_av(*avq[n_t - 1])
                return make_tail(b, qb, ctxA, ctxB, blk)

            # ---- interleaved emission --------------------------------------
            # QKV chunks interleave between attention blocks; chunks >= 2 get
            # a LATER scheduler priority (+offset) so they only fill true PE
            # bubbles (exp waits) instead of splitting the row-packed scores
            # pairs mid-flight (observed: 21/80 pairs broken -> ~12us lost)
            done_ch = 0
            pend = []
            # block order: batch 1 rotated so the final block is the small
            # 4-k-tile (1,0) -- its tail (the only unoverlapped one) and its
            # exp stretch are the shortest. `need` = chunks the block reads
            # (its queries + all its causal keys); chunks are emitted at the
            # correctness-minimum point and bumped later in priority, so they
            # fill exp-wait PE bubbles instead of racing attention.
            blocks = [(0, 0, 1), (0, 1, 2), (0, 2, 3), (0, 3, 4),
                      (1, 1, 6), (1, 2, 7), (1, 3, 8), (1, 0, 8)]
            for blk, (b, qb, need) in enumerate(blocks):
                while done_ch < need:
                    if done_ch >= 1:
                        with tc.high_priority(offset=-250):
                            emit_qkv(done_ch)
                    else:
                        emit_qkv(done_ch)
                    done_ch += 1
                pend = emit_attn(b, qb, pend, blk)
            for step in pend:
                step()

        if repeat == 1:
            body()
        else:
            with tc.For_i(0, repeat, 1):
                body()

    nc.compile()
    return nc


def _prep_inputs(x, Wq, bq, Wk, bk, Wv, bv, Wo):
    """Build the 8 per-core input maps (host-side sharding)."""
    x = np.asarray(x, dtype=np.float32)
    xT = x.reshape(BS, DM).T.astype(BF16)  # (DM, BS)
    # (chunk, 128, feat-tile, 512): xTc[ch, p, t, q] = xT[t*128+p, ch*512+q]
    xTc = np.ascontiguousarray(
        xT.reshape(KT_FEAT, 128, N_CH, Q_W).transpose(2, 1, 0, 3))

    def _w(W, sl):  # (128, feat-tile, CPC): w[p, t, m] = W[sl][m, t*128+p]
        wT = np.asarray(W, np.float32)[sl, :].T.astype(BF16)  # (DM, CPC)
        return np.ascontiguousarray(
            wT.reshape(KT_FEAT, 128, CPC).transpose(1, 0, 2))

    in_maps = []
    for c in range(NCORES):
        sl = slice(c * CPC, (c + 1) * CPC)
        in_maps.append({
            "xT": xTc,
            "wq": _w(Wq, sl),
            "wk": _w(Wk, sl),
            "wv": _w(Wv, sl),
            "wo": np.ascontiguousarray(np.asarray(Wo, np.float32)[:, sl].T).astype(BF16),
            "bq": np.asarray(bq, np.float32)[sl].reshape(CPC, 1).copy(),
        })
    return in_maps


def _run(in_maps, trace=False):
    global LAST_EXEC_NS, LAST_RESULTS
    from concourse import bass_utils

    if "nc" not in _CACHE:
        _CACHE["nc"] = _build()
    nc = _CACHE["nc"]
    res = bass_utils.run_bass_kernel_spmd(
        nc, in_maps, core_ids=list(range(NCORES)), trace=trace,
    )
    LAST_EXEC_NS = getattr(res, "exec_time_ns", None)
    LAST_RESULTS = res
    return res.results


def kernel(x, Wq, bq, Wk, bk, Wv, bv, Wo, bo, attention_mask=None, _trace=False):
    """Full inputs in, full output out. attention_mask is the reference's
    causal mask; causality is hardcoded in the kernel."""
    in_maps = _prep_inputs(x, Wq, bq, Wk, bk, Wv, bv, Wo)
    results = _run(in_maps, trace=_trace)
    acc = np.zeros((BS, DM), dtype=np.float32)
    for c in range(NCORES):
        acc += results[c]["out"].astype(np.float32)
    # bias folds: K bias is softmax-invariant (dropped); V bias passes through
    # attention unchanged (weights sum to 1) so it lands here via Wo
    bo_eff = (np.asarray(bo, np.float32)
              + np.asarray(bv, np.float32) @ np.asarray(Wo, np.float32).T)
    acc += bo_eff[None, :]
    return acc.reshape(B, S, DM)
